# revision 11
# baseline (speedup 1.0000x reference)
"""Trainium2 Bass kernel for Mistral-style attention with an INVERTED band mask.

Reference semantics (S=2048, E=4096, H=32, KV=8, D=128, WINDOW=1024):
  q/k/v projections -> RoPE(q,k) -> GQA attention where positions with
  |i-j| < 1024 are masked OUT (attend only to far positions) -> softmax ->
  out projection.

Sharding (8 cores, tensor-parallel by GQA group):
  core c owns KV head c and Q heads 4c..4c+3. Column-parallel QKV,
  row-parallel O projection; the 8 fp16 partial outputs are summed on host.

On-device layout: everything transposed so matmuls contract on partitions.
  Host passes hidden^T, fused Wqkv^T slice, Wo^T slice, RoPE tables
  (transposed, sign-folded; sin2 = sin rolled by 64 so the rotate-half can
  happen AFTER the multiply via one SBUF->SBUF DMA swap), and two 128x128
  triangular masks for the blocks straddling the |i-j|=1024 boundary.

Block sparsity: score block (bi,bj) [128x128] is computed only when
  |bi-bj| >= 8; blocks at exactly |bi-bj|=8 get a triangular mask.

Scheduling notes (v6):
  - Two HWDGE rings: Sync carries latency-critical streams (hid tiles JIT
    slot-gated with 8-deep lookahead, output rows); Scalar carries bulk
    weights + rope swaps + V transposes.
  - Phase-1 chunks [0,1,2,3]; the LAST chunk runs in two passes
    (k/v/q0/q1, then q2/q3 on reloaded hid tiles) so PSUM banks free up
    mid-chunk and the first attention chunk (c2) interleaves its
    scores->exp->AV chains under pass-B projection matmuls.
  - Main attention order [3,0,1]; O-projection matmuls of the previous
    chunk interleave into the block loop as PE filler while ScalarE runs
    exp; scores pipelined one block ahead; psa/pso share one PSUM tag.
  - RoPE multiplies emitted before any rot-dependent add so PSUM banks
    release at DVE line rate; f16 intermediates write dst directly.
  - AV / denominator matmuls restricted to the valid query range; psd
    drained by a ScalarE copy; reciprocal via reciprocal_approx_fast.
"""

import math
from contextlib import ExitStack

import numpy as np
import ml_dtypes

import concourse.bass as bass
import concourse.mybir as mybir
import concourse.tile as tile
from concourse import bacc
from concourse.bass_utils import run_bass_kernel_spmd

P = 128
S = 2048
E = 4096
D = 128
HPC = 4          # q heads per core
NE = E // P      # 32 e-tiles
NE2 = NE // 2    # 16 double-e tiles
NSCH = 4         # s-chunks of 512
SCH = S // NSCH  # 512
NST = S // P     # 16 s-tiles
NEO = 8          # output e-chunks of 512
WQKV = HPC * D + 2 * D  # 768 fused qkv weight cols per e-tile
SCALE = 1.0 / math.sqrt(D)
F16 = mybir.dt.float16
F32 = mybir.dt.float32
BF16 = mybir.dt.bfloat16

P1_ORDER = [0, 1, 2, 3]   # phase-1 chunk order (last one is two-pass)
AT_FIRST = 2              # attention chunk interleaved into phase-1 pass B
AT_MAIN = [3, 0, 1]       # main-loop attention chunk order
HID_AHEAD = 8             # hid DMA lookahead (in [128,2,512] tiles)


def _allowed_tiles(c):
    """For s-chunk c (query blocks bi=4c..4c+3), list (bj, lo, hi, mask, mpos):
    key tile bj is needed for query sub-tiles [lo, hi) (chunk-relative);
    mask in {None,'low','up'} applied at chunk-relative position mpos."""
    out = []
    bis = range(4 * c, 4 * c + 4)
    for bj in range(NST):
        ok = [bi for bi in bis if abs(bi - bj) >= 8]
        if not ok:
            continue
        lo = min(ok) - 4 * c
        hi = max(ok) + 1 - 4 * c
        assert ok == list(range(lo + 4 * c, hi + 4 * c)), (c, bj, ok)
        mask, mpos = None, 0
        if bj - 8 in ok:
            mask, mpos = "low", bj - 8 - 4 * c
        elif bj + 8 in ok:
            mask, mpos = "up", bj + 8 - 4 * c
        out.append((bj, lo, hi, mask, mpos))
    return out


def build_nc(debug=False):
    nc = bacc.Bacc("TRN2", target_bir_lowering=False, debug=False)
    hidT = nc.dram_tensor("hidT", (E, S), F16, kind="ExternalInput")
    wqkvT = nc.dram_tensor("wqkvT", (E, WQKV), F16, kind="ExternalInput")
    woT = nc.dram_tensor("woT", (HPC * D, E), F16, kind="ExternalInput")
    cosT = nc.dram_tensor("cosT", (D, S), F32, kind="ExternalInput")
    sin2T = nc.dram_tensor("sin2T", (D, S), F32, kind="ExternalInput")
    mlow = nc.dram_tensor("mlow", (P, P), BF16, kind="ExternalInput")
    mup = nc.dram_tensor("mup", (P, P), BF16, kind="ExternalInput")
    outd = nc.dram_tensor("out", (S, E), F16, kind="ExternalOutput")

    with tile.TileContext(nc) as tc, ExitStack() as ctx:
        const = ctx.enter_context(tc.tile_pool(name="const", bufs=1))

        wqkvT_r = wqkvT.rearrange("(eo p) d -> p eo d", p=P)
        woT_r = woT.rearrange("(ho p) e -> p ho e", p=P)
        hidT_r = hidT.rearrange("(eo p) s -> p eo s", p=P)

        # SBUF homes (persistent)
        qT_sb = const.tile([P, HPC, S], F16)     # Q^T per head [d, s]
        kT_sb = const.tile([P, S], F16)          # K^T [d, s]
        v_sb = const.tile([P, NST, D], F16)      # V [s-tile, d]
        attn_sb = const.tile([P, HPC, S], F16)   # attn_out^T per head [d, s]

        wqkv_t = [const.tile([P, WQKV], F16, name=f"wqkv{e}") for e in range(NE)]
        wo_t = [const.tile([P, E], F16, name=f"wo{h}") for h in range(HPC)]
        cos_sb = const.tile([P, S], F32)
        sin2_sb = const.tile([P, S], F32)
        ml_sb = const.tile([P, P], BF16)
        mu_sb = const.tile([P, P], BF16)
        ones_sb = const.tile([P, P], F16)

        def wq_ap(e, h):
            return wqkv_t[e][:, h * D:(h + 1) * D]

        def wk_ap(e):
            return wqkv_t[e][:, HPC * D:HPC * D + D]

        def wv_ap(e):
            return wqkv_t[e][:, HPC * D + D:]

        hidp = ctx.enter_context(tc.tile_pool(name="hid", bufs=HID_AHEAD))
        hid_tiles = {}

        def issue_hid_dma(c, e2, rnd=0):
            ht = hidp.tile([P, 2, SCH], F16, tag="hid")
            nc.sync.dma_start(
                ht[:], hidT_r[:, 2 * e2:2 * e2 + 2, c * SCH:(c + 1) * SCH])
            hid_tiles[(c, e2, rnd)] = ht

        # --- upfront DMA program (scalar ring for weights; sync for hid) ---
        c0 = P1_ORDER[0]
        for j in range(HID_AHEAD):
            issue_hid_dma(c0, j)
            if 2 * j + 1 < NE:
                nc.scalar.dma_start(wqkv_t[2 * j][:], wqkvT_r[:, 2 * j, :])
                nc.scalar.dma_start(wqkv_t[2 * j + 1][:], wqkvT_r[:, 2 * j + 1, :])
            if j == 0:
                nc.gpsimd.memset(ones_sb[:], 1.0)
        for e in range(2 * HID_AHEAD, NE):
            nc.scalar.dma_start(wqkv_t[e][:], wqkvT_r[:, e, :])
        # rope tables / masks: first needed at the first chunk's rope (~50us in)
        nc.scalar.dma_start(cos_sb[:], cosT[:])
        nc.scalar.dma_start(sin2_sb[:], sin2T[:])
        nc.scalar.dma_start(ml_sb[:], mlow[:])
        nc.scalar.dma_start(mu_sb[:], mup[:])

        rp = ctx.enter_context(tc.tile_pool(name="rope", bufs=2))

        def rope_chunk(tens, c):
            """For each (src_psum, dst): dst = src*cos + rot64(src*sin2).
            All multiplies emitted first so PSUM banks release at DVE line
            rate; rot swaps (scalar-ring SBUF DMAs) land while the DVE
            works; the in-place adds then never block the DVE FIFO."""
            csl = slice(c * SCH, (c + 1) * SCH)
            t2s = []
            for i, (ps, dst) in enumerate(tens):
                nc.vector.tensor_tensor(dst, ps, cos_sb[:, csl],
                                        mybir.AluOpType.mult)
                t2 = rp.tile([P, SCH], F16, tag="t2", bufs=5, name=f"t2_{i}")
                nc.vector.tensor_tensor(t2[:], ps, sin2_sb[:, csl],
                                        mybir.AluOpType.mult)
                t2s.append(t2)
            rots = []
            for i, t2 in enumerate(t2s):
                rot = rp.tile([P, SCH], F16, tag="rot", bufs=5, name=f"rot_{i}")
                nc.scalar.dma_start(rot[0:64, :], t2[64:128, :])
                nc.scalar.dma_start(rot[64:128, :], t2[0:64, :])
                rots.append(rot)
            for (ps, dst), rot in zip(tens, rots):
                nc.vector.tensor_tensor(dst, dst, rot[:], mybir.AluOpType.add)

        def drain_v(psvT, c):
            vstage = rp.tile([P, SCH], F16, tag="vstage", bufs=2)
            nc.scalar.copy(vstage[:], psvT[:])
            nc.scalar.dma_start_transpose(
                v_sb[:, c * 4:(c + 1) * 4, :], vstage[:])

        # ONE PSUM pool: every accumulator here is a [P,SCH] f32 = one
        # bank, so a single 8-slot tag serves all phases and slots recycle
        # across phase boundaries without pool-reservation conflicts.
        psp = ctx.enter_context(tc.tile_pool(name="ps", bufs=8, space="PSUM"))

        ep = ctx.enter_context(tc.tile_pool(name="expp", bufs=3))
        np_pool = ctx.enter_context(tc.tile_pool(name="normp", bufs=2))
        osp = ctx.enter_context(tc.tile_pool(name="ostage", bufs=2))

        # ---------- attention emission helpers ----------
        orows = {}      # st -> staged output half-row awaiting DMA
        fillers = []    # pending O-proj (st, eo) units for PE filler

        def emit_filler(n):
            for _ in range(n):
                if not fillers:
                    return
                st, eo = fillers.pop(0)
                pso = psp.tile([P, SCH], F32, tag="ps",
                               name=f"pso_{st}_{eo}")
                for h in range(HPC):
                    nc.tensor.matmul(
                        pso[:],
                        attn_sb[:, h, st * P:(st + 1) * P],
                        wo_t[h][:, eo * SCH:(eo + 1) * SCH],
                        start=(h == 0), stop=(h == HPC - 1))
                half = eo // (NEO // 2)
                if eo % (NEO // 2) == 0:
                    orows[st] = osp.tile([P, E // 2], F16, tag="orow",
                                         name=f"orow{st}_{half}")
                orow = orows[st]
                nc.vector.tensor_copy(
                    orow[:, (eo % (NEO // 2)) * SCH:
                         (eo % (NEO // 2) + 1) * SCH], pso[:])
                if eo % (NEO // 2) == NEO // 2 - 1:
                    nc.sync.dma_start(
                        outd[st * P:(st + 1) * P,
                             half * (E // 2):(half + 1) * (E // 2)],
                        orow[:])
                    del orows[st]

        def queue_oproj(c):
            for st in range(4 * c, 4 * c + 4):
                for eo in range(NEO):
                    fillers.append((st, eo))

        def emit_block_scores(c, h, blk):
            """Scores matmul + exp + mask for one block; returns pend."""
            idx, (bj, lo, hi, mask, mpos) = blk
            n = (hi - lo) * P
            pss = psp.tile([P, SCH], F32, tag="ps", name="pss")
            nc.tensor.matmul(
                pss[:, :n],
                kT_sb[:, bj * P:(bj + 1) * P],
                qT_sb[:, h, c * SCH + lo * P: c * SCH + hi * P],
                start=True, stop=True)
            et = ep.tile([P, SCH], BF16, tag="exp")
            nc.scalar.activation(
                et[:, lo * P:hi * P], pss[:, :n],
                mybir.ActivationFunctionType.Exp, scale=SCALE)
            if mask == "low":
                nc.vector.tensor_tensor(
                    et[:, mpos * P:(mpos + 1) * P],
                    et[:, mpos * P:(mpos + 1) * P],
                    ml_sb[:], mybir.AluOpType.mult)
            elif mask == "up":
                nc.vector.tensor_tensor(
                    et[:, mpos * P:(mpos + 1) * P],
                    et[:, mpos * P:(mpos + 1) * P],
                    mu_sb[:], mybir.AluOpType.mult)
            return (idx, bj, lo, hi, et)

        def emit_av(psa, psd, pend, nblk):
            idx, bj, lo, hi, et = pend
            sl = slice(lo * P, hi * P)
            nc.tensor.matmul(
                psa[:, sl], v_sb[:, bj, :], et[:, sl],
                start=(idx == 0), stop=(idx == nblk - 1))
            nc.tensor.matmul(
                psd[:, sl], ones_sb[:], et[:, sl],
                start=(idx == 0), stop=(idx == nblk - 1))

        def emit_norm(c, h, psa, psd):
            """Denominator drain (ScalarE), reciprocal, broadcast, multiply."""
            dsb = np_pool.tile([1, SCH], F32, tag="dsb")
            nc.scalar.copy(dsb[:], psd[0:1, :])
            rc = np_pool.tile([1, SCH], F32, tag="recip")
            nc.vector.reciprocal_approx_fast(rc[:], dsb[:])
            bc = np_pool.tile([P, SCH], F32, tag="bcast")
            nc.gpsimd.partition_broadcast(bc[:], rc[:])
            nc.vector.tensor_tensor(
                attn_sb[:, h, c * SCH:(c + 1) * SCH], psa[:], bc[:],
                mybir.AluOpType.mult)

        def gen_attn_chunk(c):
            """Generator: one yield per emission piece (block or norm),
            software-pipelined one block ahead (AV lags scores)."""
            blocks = _allowed_tiles(c)
            nblk = len(blocks)
            for h in range(HPC):
                psa = psp.tile([P, SCH], F32, tag="ps", name=f"psa{h}")
                psd = psp.tile([P, SCH], F32, tag="ps", name="psd")
                pend = None
                for blk in enumerate(blocks):
                    npend = emit_block_scores(c, h, blk)
                    if pend is not None:
                        emit_av(psa, psd, pend, nblk)
                    pend = npend
                    yield
                emit_av(psa, psd, pend, nblk)
                emit_norm(c, h, psa, psd)
                yield

        # ---- Phase 1: QKV projections (+RoPE), chunks 0..2 ----
        clast = P1_ORDER[-1]
        for ci, c in enumerate(P1_ORDER[:-1]):
            psq = [psp.tile([P, SCH], F32, tag="ps", name=f"psq{h}")
                   for h in range(HPC)]
            psk = psp.tile([P, SCH], F32, tag="ps", name="psk")
            psvT = psp.tile([P, SCH], F32, tag="ps", name="psv")
            for e2 in range(NE2):
                ht = hid_tiles.pop((c, e2, 0))
                # k/v first (lead-in work while psq banks free up)
                for j in range(2):
                    e = 2 * e2 + j
                    nc.tensor.matmul(psk[:], wk_ap(e), ht[:, j, :],
                                     start=(e == 0), stop=(e == NE - 1))
                    nc.tensor.matmul(psvT[:], wv_ap(e), ht[:, j, :],
                                     start=(e == 0), stop=(e == NE - 1))
                for j in range(2):
                    e = 2 * e2 + j
                    for h in range(HPC):
                        nc.tensor.matmul(psq[h][:], wq_ap(e, h), ht[:, j, :],
                                         start=(e == 0), stop=(e == NE - 1))
                nxt = e2 + HID_AHEAD
                if nxt < NE2:
                    issue_hid_dma(c, nxt)
                else:
                    issue_hid_dma(P1_ORDER[ci + 1], nxt - NE2)
            drain_v(psvT, c)
            tens = [(psk[:], kT_sb[:, c * SCH:(c + 1) * SCH])]
            tens += [(psq[h][:], qT_sb[:, h, c * SCH:(c + 1) * SCH])
                     for h in range(HPC)]
            rope_chunk(tens, c)
            if ci == 1:
                # bulk wo loads: needed first ~40us into attention
                for h in range(HPC):
                    nc.scalar.dma_start(wo_t[h][:], woT_r[:, h, :])

        # ---- Last chunk: pass A (k/v/q0/q1), pass B (q2/q3) with the first
        # attention chunk's block chains interleaved under its matmuls ----
        c = clast
        psq = [psp.tile([P, SCH], F32, tag="ps", name=f"psq{h}")
               for h in range(HPC)]
        psk = psp.tile([P, SCH], F32, tag="ps", name="psk")
        psvT = psp.tile([P, SCH], F32, tag="ps", name="psv")
        for e2 in range(NE2):
            ht = hid_tiles.pop((c, e2, 0))
            for j in range(2):
                e = 2 * e2 + j
                nc.tensor.matmul(psk[:], wk_ap(e), ht[:, j, :],
                                 start=(e == 0), stop=(e == NE - 1))
                nc.tensor.matmul(psvT[:], wv_ap(e), ht[:, j, :],
                                 start=(e == 0), stop=(e == NE - 1))
            for j in range(2):
                e = 2 * e2 + j
                for h in (0, 1):
                    nc.tensor.matmul(psq[h][:], wq_ap(e, h), ht[:, j, :],
                                     start=(e == 0), stop=(e == NE - 1))
            nxt = e2 + HID_AHEAD
            if nxt < NE2:
                issue_hid_dma(c, nxt)
            else:
                issue_hid_dma(c, nxt - NE2, rnd=1)  # reload for pass B
        drain_v(psvT, c)
        tens = [(psk[:], kT_sb[:, c * SCH:(c + 1) * SCH])]
        tens += [(psq[h][:], qT_sb[:, h, c * SCH:(c + 1) * SCH]) for h in (0, 1)]
        rope_chunk(tens, c)

        attn_gen = gen_attn_chunk(AT_FIRST)
        for e2 in range(NE2):
            ht = hid_tiles.pop((c, e2, 1))
            for j in range(2):
                e = 2 * e2 + j
                for h in (2, 3):
                    nc.tensor.matmul(psq[h][:], wq_ap(e, h), ht[:, j, :],
                                     start=(e == 0), stop=(e == NE - 1))
            nxt = e2 + HID_AHEAD
            if nxt < NE2:
                issue_hid_dma(c, nxt, rnd=1)
            if e2 >= 2:
                for _ in range(2):
                    next(attn_gen, None)
        for _ in attn_gen:
            pass
        tens = [(psq[h][:], qT_sb[:, h, c * SCH:(c + 1) * SCH]) for h in (2, 3)]
        rope_chunk(tens, c)
        queue_oproj(AT_FIRST)

        # ---- main attention loop with O-projection fillers ----
        for c in AT_MAIN:
            blocks = _allowed_tiles(c)
            nblk = len(blocks)
            for h in range(HPC):
                psa = psp.tile([P, SCH], F32, tag="ps", name=f"psa{h}")
                psd = psp.tile([P, SCH], F32, tag="ps", name="psd")
                pend = None
                for blk in enumerate(blocks):
                    npend = emit_block_scores(c, h, blk)
                    if pend is not None:
                        emit_av(psa, psd, pend, nblk)
                        emit_filler(1)
                    pend = npend
                emit_av(psa, psd, pend, nblk)
                emit_filler(1)
                emit_norm(c, h, psa, psd)
                emit_filler(2)
            emit_filler(len(fillers))  # drain leftovers before requeueing
            queue_oproj(c)
        emit_filler(len(fillers))
    nc.compile()
    return nc


_NC_CACHE = {}


def get_nc():
    if "nc" not in _NC_CACHE:
        _NC_CACHE["nc"] = build_nc()
    return _NC_CACHE["nc"]


def make_in_maps(hidden_states, Wq, Wk, Wv, Wo):
    hid = np.asarray(hidden_states).reshape(S, E)
    hidT16 = np.ascontiguousarray(hid.T).astype(np.float16)

    inv = 1.0 / (10000.0 ** (np.arange(0, D, 2, dtype=np.float64) / D))
    t = np.arange(S, dtype=np.float64)
    fr = np.outer(t, inv)                      # [S, 64]
    emb = np.concatenate([fr, fr], axis=1)     # [S, 128]
    cosT = np.ascontiguousarray(np.cos(emb).T).astype(np.float32)
    sinF = np.ascontiguousarray(np.sin(emb).T).astype(np.float32)
    sinF[:64] *= -1.0                          # rotate_half sign fold
    sin2T = np.ascontiguousarray(np.roll(sinF, -64, axis=0))

    jj = np.arange(P)[:, None]
    ii = np.arange(P)[None, :]
    mlow = (jj >= ii).astype(ml_dtypes.bfloat16)   # block bj-bi=8: j-i>=1024
    mup = (ii >= jj).astype(ml_dtypes.bfloat16)    # block bi-bj=8: i-j>=1024

    in_maps = []
    for c in range(8):
        qsl = slice(c * 512, (c + 1) * 512)
        ksl = slice(c * 128, (c + 1) * 128)
        wqkv = np.concatenate(
            [Wq[qsl].T, Wk[ksl].T, Wv[ksl].T], axis=1)  # [E, 768]
        in_maps.append({
            "hidT": hidT16,
            "wqkvT": np.ascontiguousarray(wqkv).astype(np.float16),
            "woT": np.ascontiguousarray(Wo[:, qsl].T).astype(np.float16),
            "cosT": cosT,
            "sin2T": sin2T,
            "mlow": mlow,
            "mup": mup,
        })
    return in_maps


def run(in_maps, **kwargs):
    nc = get_nc()
    return run_bass_kernel_spmd(nc, in_maps, core_ids=list(range(8)), **kwargs)


def kernel(hidden_states, Wq, Wk, Wv, Wo):
    in_maps = make_in_maps(hidden_states, Wq, Wk, Wv, Wo)
    res = run(in_maps)
    out = np.zeros((S, E), dtype=np.float32)
    for r in res.results:
        out += r["out"].astype(np.float32)
    return out.reshape(1, S, E)


# revision 12
# speedup vs baseline: 1.0356x; 1.0356x over previous
"""Trainium2 Bass kernel for Mistral-style attention with an INVERTED band mask.

Reference semantics (S=2048, E=4096, H=32, KV=8, D=128, WINDOW=1024):
  q/k/v projections -> RoPE(q,k) -> GQA attention where positions with
  |i-j| < 1024 are masked OUT (attend only to far positions) -> softmax ->
  out projection.

Sharding (8 cores, tensor-parallel by GQA group):
  core c owns KV head c and Q heads 4c..4c+3. Column-parallel QKV,
  row-parallel O projection; the 8 fp16 partial outputs are summed on host.

On-device layout: everything transposed so matmuls contract on partitions.
  Host passes hidden^T, fused Wqkv^T slice, Wo^T slice, RoPE tables
  (transposed, sign-folded; sin2 = sin rolled by 64 so the rotate-half can
  happen AFTER the multiply via one SBUF->SBUF DMA swap), and two 128x128
  triangular masks for the blocks straddling the |i-j|=1024 boundary.

Block sparsity: score block (bi,bj) [128x128] is computed only when
  |bi-bj| >= 8; blocks at exactly |bi-bj|=8 get a triangular mask.

Scheduling notes (v6):
  - Two HWDGE rings: Sync carries latency-critical streams (hid tiles JIT
    slot-gated with 8-deep lookahead, output rows); Scalar carries bulk
    weights + rope swaps + V transposes.
  - Phase-1 chunks [0,1,2,3]; the LAST chunk runs in two passes
    (k/v/q0/q1, then q2/q3 on reloaded hid tiles) so PSUM banks free up
    mid-chunk and the first attention chunk (c2) interleaves its
    scores->exp->AV chains under pass-B projection matmuls.
  - Main attention order [3,0,1]; O-projection matmuls of the previous
    chunk interleave into the block loop as PE filler while ScalarE runs
    exp; scores pipelined one block ahead; psa/pso share one PSUM tag.
  - RoPE multiplies emitted before any rot-dependent add so PSUM banks
    release at DVE line rate; f16 intermediates write dst directly.
  - AV / denominator matmuls restricted to the valid query range; psd
    drained by a ScalarE copy; reciprocal via reciprocal_approx_fast.
"""

import math
from contextlib import ExitStack

import numpy as np
import ml_dtypes

import concourse.bass as bass
import concourse.mybir as mybir
import concourse.tile as tile
from concourse import bacc
from concourse.bass_utils import run_bass_kernel_spmd

P = 128
S = 2048
E = 4096
D = 128
HPC = 4          # q heads per core
NE = E // P      # 32 e-tiles
NE2 = NE // 2    # 16 double-e tiles
NSCH = 4         # s-chunks of 512
SCH = S // NSCH  # 512
NST = S // P     # 16 s-tiles
NEO = 8          # output e-chunks of 512
WQKV = HPC * D + 2 * D  # 768 fused qkv weight cols per e-tile
SCALE = 1.0 / math.sqrt(D)
F16 = mybir.dt.float16
F32 = mybir.dt.float32
BF16 = mybir.dt.bfloat16

P1_ORDER = [0, 1, 2, 3]   # phase-1 chunk order (last one is two-pass)
AT_FIRST = 2              # attention chunk interleaved into phase-1 pass B
AT_MAIN = [3, 0, 1]       # main-loop attention chunk order
HID_AHEAD = 8             # hid DMA lookahead (in [128,2,512] tiles)


def _allowed_tiles(c):
    """For s-chunk c (query blocks bi=4c..4c+3), list (bj, lo, hi, mask, mpos):
    key tile bj is needed for query sub-tiles [lo, hi) (chunk-relative);
    mask in {None,'low','up'} applied at chunk-relative position mpos."""
    out = []
    bis = range(4 * c, 4 * c + 4)
    for bj in range(NST):
        ok = [bi for bi in bis if abs(bi - bj) >= 8]
        if not ok:
            continue
        lo = min(ok) - 4 * c
        hi = max(ok) + 1 - 4 * c
        assert ok == list(range(lo + 4 * c, hi + 4 * c)), (c, bj, ok)
        mask, mpos = None, 0
        if bj - 8 in ok:
            mask, mpos = "low", bj - 8 - 4 * c
        elif bj + 8 in ok:
            mask, mpos = "up", bj + 8 - 4 * c
        out.append((bj, lo, hi, mask, mpos))
    return out


def build_nc(debug=False):
    nc = bacc.Bacc("TRN2", target_bir_lowering=False, debug=False)
    hidT = nc.dram_tensor("hidT", (E, S), F16, kind="ExternalInput")
    wqkvT = nc.dram_tensor("wqkvT", (E, WQKV), F16, kind="ExternalInput")
    woT = nc.dram_tensor("woT", (HPC * D, E), F16, kind="ExternalInput")
    cosT = nc.dram_tensor("cosT", (D, S), F32, kind="ExternalInput")
    sin2T = nc.dram_tensor("sin2T", (D, S), F32, kind="ExternalInput")
    mlow = nc.dram_tensor("mlow", (P, P), BF16, kind="ExternalInput")
    mup = nc.dram_tensor("mup", (P, P), BF16, kind="ExternalInput")
    outd = nc.dram_tensor("out", (S, E), F16, kind="ExternalOutput")

    with tile.TileContext(nc) as tc, ExitStack() as ctx:
        const = ctx.enter_context(tc.tile_pool(name="const", bufs=1))

        wqkvT_r = wqkvT.rearrange("(eo p) d -> p eo d", p=P)
        woT_r = woT.rearrange("(ho p) e -> p ho e", p=P)
        hidT_r = hidT.rearrange("(eo p) s -> p eo s", p=P)

        # SBUF homes (persistent)
        qT_sb = const.tile([P, HPC, S], F16)     # Q^T per head [d, s]
        kT_sb = const.tile([P, S], F16)          # K^T [d, s]
        v_sb = const.tile([P, NST, D], F16)      # V [s-tile, d]
        attn_sb = const.tile([P, HPC, S], F16)   # attn_out^T per head [d, s]

        wqkv_t = [const.tile([P, WQKV], F16, name=f"wqkv{e}") for e in range(NE)]
        wo_t = [const.tile([P, E], F16, name=f"wo{h}") for h in range(HPC)]
        cos_sb = const.tile([P, S], F32)
        sin2_sb = const.tile([P, S], F32)
        ml_sb = const.tile([P, P], BF16)
        mu_sb = const.tile([P, P], BF16)
        ones_sb = const.tile([P, P], F16)

        def wq_ap(e, h):
            return wqkv_t[e][:, h * D:(h + 1) * D]

        def wk_ap(e):
            return wqkv_t[e][:, HPC * D:HPC * D + D]

        def wv_ap(e):
            return wqkv_t[e][:, HPC * D + D:]

        hidp = ctx.enter_context(tc.tile_pool(name="hid", bufs=HID_AHEAD))
        hid_tiles = {}

        def issue_hid_dma(c, e2, rnd=0):
            ht = hidp.tile([P, 2, SCH], F16, tag="hid")
            nc.sync.dma_start(
                ht[:], hidT_r[:, 2 * e2:2 * e2 + 2, c * SCH:(c + 1) * SCH])
            hid_tiles[(c, e2, rnd)] = ht

        # --- upfront DMA program (scalar ring for weights; sync for hid) ---
        c0 = P1_ORDER[0]
        for j in range(HID_AHEAD):
            issue_hid_dma(c0, j)
            if 2 * j + 1 < NE:
                nc.scalar.dma_start(wqkv_t[2 * j][:], wqkvT_r[:, 2 * j, :])
                nc.scalar.dma_start(wqkv_t[2 * j + 1][:], wqkvT_r[:, 2 * j + 1, :])
            if j == 0:
                nc.gpsimd.memset(ones_sb[:], 1.0)
        for e in range(2 * HID_AHEAD, NE):
            nc.scalar.dma_start(wqkv_t[e][:], wqkvT_r[:, e, :])
        # rope tables / masks: first needed at the first chunk's rope (~50us in)
        nc.scalar.dma_start(cos_sb[:], cosT[:])
        nc.scalar.dma_start(sin2_sb[:], sin2T[:])
        nc.scalar.dma_start(ml_sb[:], mlow[:])
        nc.scalar.dma_start(mu_sb[:], mup[:])

        rp = ctx.enter_context(tc.tile_pool(name="rope", bufs=2))

        def rope_chunk(tens, c, rot_eng=None):
            """For each (src_psum, dst): dst = src*cos + rot64(src*sin2).
            All multiplies emitted first so PSUM banks release at DVE line
            rate; rot swaps (scalar-ring SBUF DMAs) land while the DVE
            works; the in-place adds then never block the DVE FIFO."""
            csl = slice(c * SCH, (c + 1) * SCH)
            t2s = []
            for i, (ps, dst) in enumerate(tens):
                nc.vector.tensor_tensor(dst, ps, cos_sb[:, csl],
                                        mybir.AluOpType.mult)
                t2 = rp.tile([P, SCH], F16, tag="t2", bufs=5, name=f"t2_{i}")
                nc.vector.tensor_tensor(t2[:], ps, sin2_sb[:, csl],
                                        mybir.AluOpType.mult)
                t2s.append(t2)
            eng = rot_eng or nc.scalar
            rots = []
            for i, t2 in enumerate(t2s):
                rot = rp.tile([P, SCH], F16, tag="rot", bufs=5, name=f"rot_{i}")
                eng.dma_start(rot[0:64, :], t2[64:128, :])
                eng.dma_start(rot[64:128, :], t2[0:64, :])
                rots.append(rot)
            for (ps, dst), rot in zip(tens, rots):
                nc.vector.tensor_tensor(dst, dst, rot[:], mybir.AluOpType.add)

        def drain_v(psvT, c, eng=None):
            vstage = rp.tile([P, SCH], F16, tag="vstage", bufs=2)
            nc.scalar.copy(vstage[:], psvT[:])
            (eng or nc.scalar).dma_start_transpose(
                v_sb[:, c * 4:(c + 1) * 4, :], vstage[:])

        # Shared PSUM pool for phase 1 + the interleaved first attention
        # chunk: every accumulator is a [P,SCH] f32 = one bank, so a single
        # 8-slot tag lets attention tiles recycle projection banks mid-pass.
        # The main attention phase gets dedicated pools (opened later) to
        # avoid slot-FIFO coupling between unrelated tile streams.
        psp_cm = tc.tile_pool(name="ps", bufs=8, space="PSUM")
        psp = psp_cm.__enter__()

        ep = ctx.enter_context(tc.tile_pool(name="expp", bufs=3))
        np_pool = ctx.enter_context(tc.tile_pool(name="normp", bufs=2))
        osp = ctx.enter_context(tc.tile_pool(name="ostage", bufs=2))

        # ---------- attention emission helpers ----------
        orows = {}      # st -> staged output half-row awaiting DMA
        fillers = []    # pending O-proj (st, eo) units for PE filler

        def emit_filler(n):
            for _ in range(n):
                if not fillers:
                    return
                st, eo = fillers.pop(0)
                pso = acc_pool.tile([P, SCH], F32, tag="acc",
                                    name=f"pso_{st}_{eo}")
                for h in range(HPC):
                    nc.tensor.matmul(
                        pso[:],
                        attn_sb[:, h, st * P:(st + 1) * P],
                        wo_t[h][:, eo * SCH:(eo + 1) * SCH],
                        start=(h == 0), stop=(h == HPC - 1))
                half = eo // (NEO // 2)
                if eo % (NEO // 2) == 0:
                    orows[st] = osp.tile([P, E // 2], F16, tag="orow",
                                         name=f"orow{st}_{half}")
                orow = orows[st]
                nc.vector.tensor_copy(
                    orow[:, (eo % (NEO // 2)) * SCH:
                         (eo % (NEO // 2) + 1) * SCH], pso[:])
                if eo % (NEO // 2) == NEO // 2 - 1:
                    nc.sync.dma_start(
                        outd[st * P:(st + 1) * P,
                             half * (E // 2):(half + 1) * (E // 2)],
                        orow[:])
                    del orows[st]

        def queue_oproj(c):
            for st in range(4 * c, 4 * c + 4):
                for eo in range(NEO):
                    fillers.append((st, eo))

        def emit_block_scores(c, h, blk, pools):
            """Scores matmul + exp + mask for one block; returns pend."""
            idx, (bj, lo, hi, mask, mpos) = blk
            n = (hi - lo) * P
            pss = pools[0]()
            nc.tensor.matmul(
                pss[:, :n],
                kT_sb[:, bj * P:(bj + 1) * P],
                qT_sb[:, h, c * SCH + lo * P: c * SCH + hi * P],
                start=True, stop=True)
            et = ep.tile([P, SCH], BF16, tag="exp")
            nc.scalar.activation(
                et[:, lo * P:hi * P], pss[:, :n],
                mybir.ActivationFunctionType.Exp, scale=SCALE)
            if mask == "low":
                nc.vector.tensor_tensor(
                    et[:, mpos * P:(mpos + 1) * P],
                    et[:, mpos * P:(mpos + 1) * P],
                    ml_sb[:], mybir.AluOpType.mult)
            elif mask == "up":
                nc.vector.tensor_tensor(
                    et[:, mpos * P:(mpos + 1) * P],
                    et[:, mpos * P:(mpos + 1) * P],
                    mu_sb[:], mybir.AluOpType.mult)
            return (idx, bj, lo, hi, et)

        def emit_av(psa, psd, pend, nblk):
            idx, bj, lo, hi, et = pend
            sl = slice(lo * P, hi * P)
            nc.tensor.matmul(
                psa[:, sl], v_sb[:, bj, :], et[:, sl],
                start=(idx == 0), stop=(idx == nblk - 1))
            nc.tensor.matmul(
                psd[:, sl], ones_sb[:], et[:, sl],
                start=(idx == 0), stop=(idx == nblk - 1))

        def emit_norm(c, h, psa, psd):
            """Denominator drain (ScalarE), reciprocal, broadcast, multiply."""
            dsb = np_pool.tile([1, SCH], F32, tag="dsb")
            nc.scalar.copy(dsb[:], psd[0:1, :])
            rc = np_pool.tile([1, SCH], F32, tag="recip")
            nc.vector.reciprocal_approx_fast(rc[:], dsb[:])
            bc = np_pool.tile([P, SCH], F32, tag="bcast")
            nc.gpsimd.partition_broadcast(bc[:], rc[:])
            nc.vector.tensor_tensor(
                attn_sb[:, h, c * SCH:(c + 1) * SCH], psa[:], bc[:],
                mybir.AluOpType.mult)

        def gen_attn_chunk(c, pools):
            """Generator: one yield per emission piece (block or norm),
            software-pipelined one block ahead (AV lags scores)."""
            blocks = _allowed_tiles(c)
            nblk = len(blocks)
            for h in range(HPC):
                psa = pools[1](h)
                psd = pools[2]()
                pend = None
                for blk in enumerate(blocks):
                    npend = emit_block_scores(c, h, blk, pools)
                    if pend is not None:
                        emit_av(psa, psd, pend, nblk)
                    pend = npend
                    yield
                emit_av(psa, psd, pend, nblk)
                emit_norm(c, h, psa, psd)
                yield

        # ---- Phase 1: QKV projections (+RoPE), chunks 0..2 ----
        clast = P1_ORDER[-1]
        for ci, c in enumerate(P1_ORDER[:-1]):
            psq = [psp.tile([P, SCH], F32, tag="ps", name=f"psq{h}")
                   for h in range(HPC)]
            psk = psp.tile([P, SCH], F32, tag="ps", name="psk")
            psvT = psp.tile([P, SCH], F32, tag="ps", name="psv")
            for e2 in range(NE2):
                ht = hid_tiles.pop((c, e2, 0))
                # k/v first (lead-in work while psq banks free up)
                for j in range(2):
                    e = 2 * e2 + j
                    nc.tensor.matmul(psk[:], wk_ap(e), ht[:, j, :],
                                     start=(e == 0), stop=(e == NE - 1))
                    nc.tensor.matmul(psvT[:], wv_ap(e), ht[:, j, :],
                                     start=(e == 0), stop=(e == NE - 1))
                for j in range(2):
                    e = 2 * e2 + j
                    for h in range(HPC):
                        nc.tensor.matmul(psq[h][:], wq_ap(e, h), ht[:, j, :],
                                         start=(e == 0), stop=(e == NE - 1))
                nxt = e2 + HID_AHEAD
                if nxt < NE2:
                    issue_hid_dma(c, nxt)
                else:
                    issue_hid_dma(P1_ORDER[ci + 1], nxt - NE2)
            drain_v(psvT, c)
            tens = [(psk[:], kT_sb[:, c * SCH:(c + 1) * SCH])]
            tens += [(psq[h][:], qT_sb[:, h, c * SCH:(c + 1) * SCH])
                     for h in range(HPC)]
            rope_chunk(tens, c)
            if ci == 1:
                # bulk wo loads: needed first ~40us into attention
                for h in range(HPC):
                    nc.scalar.dma_start(wo_t[h][:], woT_r[:, h, :])

        # ---- Last chunk: pass A (k/v/q0/q1), pass B (q2/q3) with the first
        # attention chunk's block chains interleaved under its matmuls ----
        c = clast
        psq = [psp.tile([P, SCH], F32, tag="ps", name=f"psq{h}")
               for h in range(HPC)]
        psk = psp.tile([P, SCH], F32, tag="ps", name="psk")
        psvT = psp.tile([P, SCH], F32, tag="ps", name="psv")
        for e2 in range(NE2):
            ht = hid_tiles.pop((c, e2, 0))
            for j in range(2):
                e = 2 * e2 + j
                nc.tensor.matmul(psk[:], wk_ap(e), ht[:, j, :],
                                 start=(e == 0), stop=(e == NE - 1))
                nc.tensor.matmul(psvT[:], wv_ap(e), ht[:, j, :],
                                 start=(e == 0), stop=(e == NE - 1))
            for j in range(2):
                e = 2 * e2 + j
                for h in (0, 1):
                    nc.tensor.matmul(psq[h][:], wq_ap(e, h), ht[:, j, :],
                                     start=(e == 0), stop=(e == NE - 1))
            nxt = e2 + HID_AHEAD
            if nxt < NE2:
                issue_hid_dma(c, nxt)
            else:
                issue_hid_dma(c, nxt - NE2, rnd=1)  # reload for pass B
        drain_v(psvT, c, eng=nc.sync)
        tens = [(psk[:], kT_sb[:, c * SCH:(c + 1) * SCH])]
        tens += [(psq[h][:], qT_sb[:, h, c * SCH:(c + 1) * SCH]) for h in (0, 1)]
        rope_chunk(tens, c, rot_eng=nc.sync)

        psp_pools = (
            lambda: psp.tile([P, SCH], F32, tag="ps", name="pss"),
            lambda h: psp.tile([P, SCH], F32, tag="ps", name=f"psa{h}"),
            lambda: psp.tile([P, SCH], F32, tag="ps", name="psd"),
        )
        attn_gen = gen_attn_chunk(AT_FIRST, psp_pools)
        for e2 in range(NE2):
            ht = hid_tiles.pop((c, e2, 1))
            for j in range(2):
                e = 2 * e2 + j
                for h in (2, 3):
                    nc.tensor.matmul(psq[h][:], wq_ap(e, h), ht[:, j, :],
                                     start=(e == 0), stop=(e == NE - 1))
            nxt = e2 + HID_AHEAD
            if nxt < NE2:
                issue_hid_dma(c, nxt, rnd=1)
            if e2 >= 2:
                for _ in range(2):
                    next(attn_gen, None)
        for _ in attn_gen:
            pass
        tens = [(psq[h][:], qT_sb[:, h, c * SCH:(c + 1) * SCH]) for h in (2, 3)]
        rope_chunk(tens, c, rot_eng=nc.sync)
        queue_oproj(AT_FIRST)
        psp_cm.__exit__(None, None, None)

        pss_pool = ctx.enter_context(
            tc.tile_pool(name="apss", bufs=3, space="PSUM"))
        acc_pool = ctx.enter_context(
            tc.tile_pool(name="aacc", bufs=4, space="PSUM"))
        psd_pool = ctx.enter_context(
            tc.tile_pool(name="apsd", bufs=1, space="PSUM"))
        main_pools = (
            lambda: pss_pool.tile([P, SCH], F32, tag="pss", name="pss"),
            lambda h: acc_pool.tile([P, SCH], F32, tag="acc", name=f"psa{h}"),
            lambda: psd_pool.tile([P, SCH], F32, tag="psd", name="psd"),
        )

        # ---- main attention loop with O-projection fillers ----
        for c in AT_MAIN:
            blocks = _allowed_tiles(c)
            nblk = len(blocks)
            for h in range(HPC):
                psa = main_pools[1](h)
                psd = main_pools[2]()
                pend = None
                for blk in enumerate(blocks):
                    npend = emit_block_scores(c, h, blk, main_pools)
                    if pend is not None:
                        emit_av(psa, psd, pend, nblk)
                        emit_filler(1)
                    pend = npend
                emit_av(psa, psd, pend, nblk)
                emit_filler(1)
                emit_norm(c, h, psa, psd)
                emit_filler(2)
            emit_filler(len(fillers))  # drain leftovers before requeueing
            queue_oproj(c)
        emit_filler(len(fillers))
    nc.compile()
    return nc


_NC_CACHE = {}


def get_nc():
    if "nc" not in _NC_CACHE:
        _NC_CACHE["nc"] = build_nc()
    return _NC_CACHE["nc"]


def make_in_maps(hidden_states, Wq, Wk, Wv, Wo):
    hid = np.asarray(hidden_states).reshape(S, E)
    hidT16 = np.ascontiguousarray(hid.T).astype(np.float16)

    inv = 1.0 / (10000.0 ** (np.arange(0, D, 2, dtype=np.float64) / D))
    t = np.arange(S, dtype=np.float64)
    fr = np.outer(t, inv)                      # [S, 64]
    emb = np.concatenate([fr, fr], axis=1)     # [S, 128]
    cosT = np.ascontiguousarray(np.cos(emb).T).astype(np.float32)
    sinF = np.ascontiguousarray(np.sin(emb).T).astype(np.float32)
    sinF[:64] *= -1.0                          # rotate_half sign fold
    sin2T = np.ascontiguousarray(np.roll(sinF, -64, axis=0))

    jj = np.arange(P)[:, None]
    ii = np.arange(P)[None, :]
    mlow = (jj >= ii).astype(ml_dtypes.bfloat16)   # block bj-bi=8: j-i>=1024
    mup = (ii >= jj).astype(ml_dtypes.bfloat16)    # block bi-bj=8: i-j>=1024

    in_maps = []
    for c in range(8):
        qsl = slice(c * 512, (c + 1) * 512)
        ksl = slice(c * 128, (c + 1) * 128)
        wqkv = np.concatenate(
            [Wq[qsl].T, Wk[ksl].T, Wv[ksl].T], axis=1)  # [E, 768]
        in_maps.append({
            "hidT": hidT16,
            "wqkvT": np.ascontiguousarray(wqkv).astype(np.float16),
            "woT": np.ascontiguousarray(Wo[:, qsl].T).astype(np.float16),
            "cosT": cosT,
            "sin2T": sin2T,
            "mlow": mlow,
            "mup": mup,
        })
    return in_maps


def run(in_maps, **kwargs):
    nc = get_nc()
    return run_bass_kernel_spmd(nc, in_maps, core_ids=list(range(8)), **kwargs)


def kernel(hidden_states, Wq, Wk, Wv, Wo):
    in_maps = make_in_maps(hidden_states, Wq, Wk, Wv, Wo)
    res = run(in_maps)
    out = np.zeros((S, E), dtype=np.float32)
    for r in res.results:
        out += r["out"].astype(np.float32)
    return out.reshape(1, S, E)
